# revision 1
# baseline (speedup 1.0000x reference)
"""Causal self-attention (B=4, T=4096, C=128) on 8 trn2 NeuronCores.

Sharding: core c -> (batch b=c//2, key-parity class h=c%2).
Each core processes ALL queries of its batch against the key chunks
j === h (mod 2) (128-wide chunks) -> exactly half the causal work per
core, identical instruction stream on every core (SPMD-uniform; only
the input DATA differs per core). Each core emits the unnormalized
partial attention output ou = w~^T V restricted to its key class and
the partial softmax denominators se; the host combines
  out[b] = (ou_h0 + ou_h1) / (se_h0 + se_h1).

Device math per query block (512 queries), all in "transposed score"
domain so no on-device transposes are needed (all matmuls are N=512
float32r, 1 cycle/row on the PE):
  Y^T  = matmul(lhsT=Wq^T Wk (host-fused), rhs=x^T)   [c, q]  (Y = Q Wk)
  S^T  = matmul(lhsT=xk^T chunk, rhs=Y^T)             [s, q]  (scores^T)
  w~   = exp(S^T / sqrt(C)) * causal_mask             [s, q]
  row  += matmul(lhsT=ones, rhs=w~ (chunk pairs       [1, q]  (sumexp)
          pre-summed on the vector engine))
  u    += matmul(lhsT=xk chunk, rhs=w~)               [c, q]  (Xk^T w~^T)
  ou^T = matmul(lhsT=Wv^T, rhs=u)                     [c, q]  (unnormalized)
"""

import math

import numpy as np

import concourse.mybir as mybir
import concourse.tile as tile
from concourse import bacc
from concourse.bass_utils import run_bass_kernel_spmd

B, T, C = 4, 4096, 128
P = 128            # partition width / head dim / key chunk
QB = 512           # query block (matmul free dim)
NQB = T // QB      # 8 query blocks
NCH = T // P // 2  # 16 key chunks per parity class

# dtype for matmul inputs (float32r = 4x matmul throughput vs float32)
MDT = mybir.dt.float32r

F32 = mybir.dt.float32


def build_kernel(cfg=None):
    base = dict(
        w_bufs=9, s_bufs=4, u_bufs=2, o_bufs=1, row_bufs=1,
        ws_bufs=2, usb_bufs=2, osb_bufs=4, se_bufs=2,
    )
    base.update(cfg or {})
    cfg = base
    nc = bacc.Bacc(None, target_bir_lowering=False)

    # Inputs (per-core data; identical shapes/names on every core).
    xT = nc.dram_tensor("xT", [P, T], MDT, kind="ExternalInput")      # x[b].T
    xkT = nc.dram_tensor("xkT", [P, NCH * P], MDT, kind="ExternalInput")
    xk = nc.dram_tensor("xk", [NCH * P, P], MDT, kind="ExternalInput")
    wqk = nc.dram_tensor("wqk", [P, P], MDT, kind="ExternalInput")    # Wq.T @ Wk
    wv_t = nc.dram_tensor("wv_t", [P, P], MDT, kind="ExternalInput")  # Wv.T
    mask_lo = nc.dram_tensor("mask_lo", [P, QB], MDT, kind="ExternalInput")
    mask_hi = nc.dram_tensor("mask_hi", [P, QB], MDT, kind="ExternalInput")
    ones = nc.dram_tensor("ones", [P, 1], MDT, kind="ExternalInput")

    # Outputs (ou is stored transposed: [C, T])
    ou = nc.dram_tensor("ou", [P, T], F32, kind="ExternalOutput")
    se = nc.dram_tensor("se", [NQB, QB], F32, kind="ExternalOutput")

    scale = 1.0 / math.sqrt(C)

    with tile.TileContext(nc) as tc:
        with (
            tc.tile_pool(name="const", bufs=1) as const,
            tc.tile_pool(name="wpool", bufs=cfg["w_bufs"]) as wpool,
            tc.tile_pool(name="upool", bufs=cfg["usb_bufs"]) as upool,
            tc.tile_pool(name="wspool", bufs=cfg["ws_bufs"]) as wspool,
            tc.tile_pool(name="opool", bufs=cfg["osb_bufs"]) as opool,
            tc.tile_pool(name="spool", bufs=cfg["se_bufs"]) as spool,
            tc.tile_pool(name="ps_s", bufs=cfg["s_bufs"], space="PSUM") as ps_s,
            tc.tile_pool(name="ps_row", bufs=cfg["row_bufs"], space="PSUM") as ps_row,
            tc.tile_pool(name="ps_u", bufs=cfg["u_bufs"], space="PSUM") as ps_u,
            tc.tile_pool(name="ps_o", bufs=cfg["o_bufs"], space="PSUM") as ps_o,
        ):
            # ---- load constants / activations ----
            # Small constants first: the HWDGE generates descriptors in
            # issue order, so anything the first matmuls need must go first.
            wqk_sb = const.tile([P, P], MDT)
            wv_t_sb = const.tile([P, P], MDT)
            ml_sb = const.tile([P, QB], MDT)
            mh_sb = const.tile([P, QB], MDT)
            ones_sb = const.tile([P, 1], MDT)
            xT_sb = const.tile([P, T], MDT)
            xkT_sb = const.tile([P, NCH * P], MDT)
            xk_sb = const.tile([P, NCH * P], MDT)

            # DMA issue order == descriptor-generation order. The HWDGE is
            # ONE shared unit for the sync+scalar queues (~0.63us per
            # dma_start, serialized); SWDGE (gpsimd/Pool) is independent.
            # HWDGE: critical path first (wqk, xT7), then key-chunk groups
            # and remaining xT blocks in consumption order (qblocks 7->0).
            # SWDGE: ones, xk groups, masks, wv.
            nc.sync.dma_start(wqk_sb[:], wqk[:])
            nc.gpsimd.dma_start(
                xT_sb[:, (NQB - 1) * QB :], xT[:, (NQB - 1) * QB :]
            )
            nc.sync.dma_start(
                xT_sb[:, (NQB - 2) * QB : (NQB - 1) * QB],
                xT[:, (NQB - 2) * QB : (NQB - 1) * QB],
            )
            nc.gpsimd.dma_start(ones_sb[:], ones[:])
            for g in range(0, NCH, 4):
                gs = slice(g * P, (g + 4) * P)
                nc.gpsimd.dma_start(
                    xk_sb[:, gs].rearrange("p (g c) -> p g c", g=4),
                    xk[gs, :].rearrange("(g p) c -> p g c", p=P),
                )
            nc.gpsimd.dma_start(ml_sb[:], mask_lo[:])
            nc.gpsimd.dma_start(mh_sb[:], mask_hi[:])
            nc.gpsimd.dma_start(wv_t_sb[:], wv_t[:])

            order = cfg.get("order") or [7, 6, 1, 5, 0, 4, 3, 2]
            gs0 = slice(0, 4 * P)
            nc.sync.dma_start(xkT_sb[:, gs0], xkT[:, gs0])
            xts = [n for n in order[1:] if n not in (NQB - 1, NQB - 2)]
            for g in range(4, NCH, 4):
                gs = slice(g * P, (g + 4) * P)
                nc.sync.dma_start(xkT_sb[:, gs], xkT[:, gs])
                if xts:
                    n = xts.pop(0)
                    nc.sync.dma_start(
                        xT_sb[:, n * QB : (n + 1) * QB],
                        xT[:, n * QB : (n + 1) * QB],
                    )
            for n in xts:
                nc.sync.dma_start(
                    xT_sb[:, n * QB : (n + 1) * QB], xT[:, n * QB : (n + 1) * QB]
                )

            # ---- attention per query block ----
            # Per-qblock head (Y^T projection) and epilogue (u/se
            # evacuation + Wv projection) are interleaved into the
            # surrounding qblocks' chunk streams so the PE keeps busy.
            y_all = const.tile([P, T], MDT)

            def emit_head(i):
                qs = slice(i * QB, (i + 1) * QB)
                ps = ps_s.tile([P, QB], F32, tag="ps")
                nc.tensor.matmul(ps[:], wqk_sb[:], xT_sb[:, qs], start=True, stop=True)
                nc.vector.tensor_copy(out=y_all[:, qs], in_=ps[:])

            def make_tail(i, psu, psr, final=False):
                def tail():
                    qs = slice(i * QB, (i + 1) * QB)
                    se_sb = spool.tile([1, QB], F32)
                    nc.vector.tensor_copy(out=se_sb[:], in_=psr[:])
                    nc.sync.dma_start(se[i : i + 1, :], se_sb[:])
                    u_sb = upool.tile([P, QB], MDT)
                    pso = ps_o.tile([P, QB], F32)
                    o_sb = opool.tile([P, QB], F32)
                    if not final:
                        nc.vector.tensor_copy(out=u_sb[:], in_=psu[:])
                        nc.tensor.matmul(
                            pso[:], wv_t_sb[:], u_sb[:], start=True, stop=True
                        )
                        nc.vector.tensor_copy(out=o_sb[:], in_=pso[:])
                        nc.sync.dma_start(ou[:, qs], o_sb[:])
                    else:
                        # Final epilogue: nothing left to hide behind, so
                        # pipeline it in half-width pieces across queues.
                        H = QB // 2
                        for k in range(2):
                            hs = slice(k * H, (k + 1) * H)
                            ds = slice(i * QB + k * H, i * QB + (k + 1) * H)
                            nc.vector.tensor_copy(out=u_sb[:, hs], in_=psu[:, hs])
                            nc.tensor.matmul(
                                pso[:, hs], wv_t_sb[:], u_sb[:, hs],
                                start=True, stop=True,
                            )
                            nc.vector.tensor_copy(out=o_sb[:, hs], in_=pso[:, hs])
                            q_eng = nc.sync if k == 0 else nc.scalar
                            q_eng.dma_start(ou[:, ds], o_sb[:, hs])

                return tail

            def emit_last_accum(psu_t, psr_t, nch_, wt):
                # accum for a qblock's final (restricted) chunk; explicit
                # args because the loop locals are rebound across qblocks
                c = nch_ - 1
                cs = slice(c * P, (c + 1) * P)
                nc.tensor.matmul(
                    psr_t[:, 256:], ones_sb[:], wt[:, 256:],
                    start=False, stop=True,
                )
                nc.tensor.matmul(
                    psu_t[:, 256:], xk_sb[:, cs], wt[:, 256:],
                    start=False, stop=True,
                )

            pending_tail = None
            pending_accum = None
            heads = list(order)
            emit_head(heads.pop(0))
            emit_head(heads.pop(0))
            for oi, i in enumerate(order):
                nch = 2 * (i + 1)
                ysb = y_all[:, i * QB : (i + 1) * QB]

                psu = ps_u.tile([P, QB], F32)
                psr = ps_row.tile([1, QB], F32)

                def emit_score(c):
                    # Final (diagonal) chunk: queries < 256 are entirely
                    # before this key chunk for both parities -> compute
                    # only columns [256, 512).
                    o = 256 if c == nch - 1 else 0
                    cs = slice(c * P, (c + 1) * P)
                    pss = ps_s.tile([P, QB], F32, tag="ps")
                    nc.tensor.matmul(
                        pss[:, o:], xkT_sb[:, cs], ysb[:, o:], start=True, stop=True
                    )
                    wt = wpool.tile([P, QB], MDT)
                    nc.scalar.activation(
                        wt[:, o:], pss[:, o:], mybir.ActivationFunctionType.Exp,
                        scale=scale,
                    )
                    if c == nch - 2:
                        nc.vector.tensor_mul(
                            out=wt[:, 0:256], in0=wt[:, 0:256], in1=ml_sb[:, 0:256]
                        )
                    elif c == nch - 1:
                        nc.vector.tensor_mul(
                            out=wt[:, 256:], in0=wt[:, 256:], in1=mh_sb[:, 256:]
                        )
                    return wt

                w_stash = []

                def emit_accum(c, wt):
                    o = 256 if c == nch - 1 else 0
                    cs = slice(c * P, (c + 1) * P)
                    first, last = c == 0, c == nch - 1
                    # psr (sumexp) uses the same lhsT for every chunk, so
                    # chunk pairs are pre-summed on DVE and streamed
                    # through the PE once. The final two chunks (mask /
                    # restricted columns) stay individual.
                    if c < nch - 2:
                        if not w_stash:
                            w_stash.append((c, wt))
                        else:
                            c0, wt0 = w_stash.pop()
                            ws = wspool.tile([P, QB], MDT)
                            nc.vector.tensor_add(out=ws[:], in0=wt0[:], in1=wt[:])
                            nc.tensor.matmul(
                                psr[:], ones_sb[:], ws[:],
                                start=(c0 == 0), stop=False,
                            )
                    else:
                        nc.tensor.matmul(
                            psr[:, o:], ones_sb[:], wt[:, o:],
                            start=first, stop=last,
                        )
                    nc.tensor.matmul(
                        psu[:, o:], xk_sb[:, cs], wt[:, o:], start=first, stop=last
                    )

                # software-pipeline by one chunk; the previous qblock's
                # LAST accum, its epilogue, and the next qblock's head are
                # all deferred into this qblock's chunk stream so the PE
                # never waits on the exp->mask chain at a boundary.
                wt_prev = emit_score(0)
                if pending_accum is not None:
                    pending_accum()
                    pending_accum = None
                for c in range(1, nch):
                    wt_c = emit_score(c)
                    emit_accum(c - 1, wt_prev)
                    wt_prev = wt_c
                    if c == 1 and pending_tail is not None:
                        pending_tail()
                        pending_tail = None
                    c_head = 1 if nch == 2 else max(2, nch - 4)
                    if c == c_head and heads:
                        emit_head(heads.pop(0))
                pending_accum = (
                    lambda pu=psu, pr=psr, n=nch, w=wt_prev: emit_last_accum(
                        pu, pr, n, w
                    )
                )
                if pending_tail is not None:  # nch == 2 case
                    pending_tail()
                pending_tail = make_tail(i, psu, psr, final=oi == NQB - 1)
            pending_accum()
            pending_tail()

    nc.compile()
    return nc


_NC_CACHE = {}


def _get_nc():
    if "nc" not in _NC_CACHE:
        _NC_CACHE["nc"] = build_kernel()
    return _NC_CACHE["nc"]


_STATIC = {}


def _static_parts(h):
    if h not in _STATIC:
        rows = np.concatenate(
            [np.arange(j * P, (j + 1) * P) for j in range(h, T // P, 2)]
        )
        s = np.arange(P)[:, None]
        q = np.arange(QB)[None, :]
        _STATIC[h] = (
            rows,
            (q >= s + P * h).astype(np.float32),
            (q >= s + P * (h + 2)).astype(np.float32),
            np.ones((P, 1), dtype=np.float32),
        )
    return _STATIC[h]


def _core_inputs(xb, Wq, Wk, Wv, h):
    """Build the input map for one core (batch data xb [T,C], parity h)."""
    rows, mask_lo, mask_hi, ones_arr = _static_parts(h)
    xk = np.ascontiguousarray(xb[rows])            # [NCH*P, C]
    return {
        "xT": np.ascontiguousarray(xb.T),
        "xkT": np.ascontiguousarray(xk.T),
        "xk": xk,
        "wqk": np.ascontiguousarray(Wq.T @ Wk),
        "wv_t": np.ascontiguousarray(Wv.T),
        "mask_lo": mask_lo,
        "mask_hi": mask_hi,
        "ones": ones_arr,
    }


def _build_runner(nc):
    """Cacheable PJRT runner (same machinery as bass2jax.run_bass_via_pjrt,
    but the jitted executable is built once and reused across kernel()
    calls instead of being re-traced every time)."""
    import jax
    from jax.sharding import Mesh, PartitionSpec
    from jax.experimental.shard_map import shard_map
    from concourse.bass2jax import (
        _bass_exec_p, install_neuronx_cc_hook, partition_id_tensor,
    )

    install_neuronx_cc_hook()
    pname = nc.partition_id_tensor.name if nc.partition_id_tensor else None
    in_names, out_names, out_avals, out_shapes = [], [], [], []
    for alloc in nc.m.functions[0].allocations:
        if not isinstance(alloc, mybir.MemoryLocationSet):
            continue
        name = alloc.memorylocations[0].name
        if alloc.kind == "ExternalInput":
            if name != pname:
                in_names.append(name)
        elif alloc.kind == "ExternalOutput":
            shape = tuple(alloc.tensor_shape)
            dtype = mybir.dt.np(alloc.dtype)
            out_names.append(name)
            out_avals.append(jax.core.ShapedArray(shape, dtype))
            out_shapes.append((shape, dtype))
    n_params, n_outs = len(in_names), len(out_avals)
    all_in = in_names + out_names + ([pname] if pname else [])
    donate = tuple(range(n_params, n_params + n_outs))

    def _body(*args):
        operands = list(args)
        if pname is not None:
            operands.append(partition_id_tensor())
        return tuple(
            _bass_exec_p.bind(
                *operands,
                out_avals=tuple(out_avals),
                in_names=tuple(all_in),
                out_names=tuple(out_names),
                lowering_input_output_aliases=(),
                sim_require_finite=True,
                sim_require_nnan=True,
                nc=nc,
            )
        )

    devices = jax.devices()[:8]
    mesh = Mesh(np.asarray(devices), ("core",))
    sharded = jax.jit(
        shard_map(
            _body, mesh=mesh,
            in_specs=(PartitionSpec("core"),) * (n_params + n_outs),
            out_specs=(PartitionSpec("core"),) * n_outs,
            check_rep=False,
        ),
        donate_argnums=donate, keep_unused=True,
    )

    def run(in_maps):
        concat_in = [
            np.concatenate([np.asarray(m[nm]) for m in in_maps], axis=0)
            for nm in in_names
        ]
        zeros = [
            np.zeros((8 * s[0],) + s[1:], d) for s, d in out_shapes
        ]
        outs = sharded(*concat_in, *zeros)
        return [
            {
                nm: np.asarray(outs[j]).reshape(8, *out_shapes[j][0])[c]
                for j, nm in enumerate(out_names)
            }
            for c in range(8)
        ]

    return run


def kernel(x, Wq, Wk, Wv, _trace=False):
    x = np.asarray(x, dtype=np.float32)
    Wq = np.asarray(Wq, dtype=np.float32)
    Wk = np.asarray(Wk, dtype=np.float32)
    Wv = np.asarray(Wv, dtype=np.float32)

    nc = _get_nc()
    in_maps = [_core_inputs(x[c // 2], Wq, Wk, Wv, c % 2) for c in range(8)]
    results = None
    if not _trace:
        try:
            if "runner" not in _NC_CACHE:
                _NC_CACHE["runner"] = _build_runner(nc)
            results = _NC_CACHE["runner"](in_maps)
        except Exception:
            _NC_CACHE.pop("runner", None)
            results = None
    if results is None:
        try:
            res = run_bass_kernel_spmd(
                nc, in_maps, core_ids=list(range(8)), trace=_trace
            )
        except ModuleNotFoundError:
            # axon NTFF profiling hook unavailable in this container
            res = run_bass_kernel_spmd(nc, in_maps, core_ids=list(range(8)))
        if _trace:
            _NC_CACHE["last_results"] = res
        results = res.results

    out = np.empty((B, T, C), dtype=np.float32)
    for b in range(B):
        a, bb = results[2 * b], results[2 * b + 1]
        denom = a["se"].reshape(T) + bb["se"].reshape(T)
        out[b] = ((a["ou"] + bb["ou"]) / denom[None, :]).T
    return out



# revision 36
# speedup vs baseline: 1.2234x; 1.2234x over previous
"""Causal self-attention (B=4, T=4096, C=128) on 8 trn2 NeuronCores.

Sharding: core c -> (batch b=c//2, key-parity class h=c%2).
Each core processes ALL queries of its batch against the key chunks
j === h (mod 2) (128-wide chunks) -> exactly half the causal work per
core, identical instruction stream on every core (SPMD-uniform; only
the input DATA differs per core). Each core emits the unnormalized
partial attention output ou = w~^T V restricted to its key class and
the partial softmax denominators se; the host combines
  out[b] = (ou_h0 + ou_h1) / (se_h0 + se_h1).

Engine plan (per 512-query block, all SBUF data bf16):
  PE : Z = (Wk^T Wq) Xk^T once per key chunk (key-side transform, so
       scores consume raw xT straight from the input tile);
       V chunks = Xk Wv^T once;
       S^T chunk = Z_c^T X^T  (into 3-bank PSUM tiles, 3 chunks each);
       ou^T += V_c^T w~_c (PSUM accumulation across chunks).
  Act: one Exp activation per 3-chunk PSUM tile (packing amortizes the
       ~185ns access overhead) writing bf16 into a wide per-qblock w
       tile.  Act is the roofline engine (~34us of exp columns).
  DVE: causal-mask multiplies + running sum of w~ chunks (bf16 2x mode).
  Pool: PSUM->SBUF evacuations (z, v, ou) + partition_all_reduce of the
       summed w~ tile -> softmax denominators (PE does no sumexp work).
All inputs ride ONE packed DRAM tensor laid out in consumption order so
a few descending-priority DMAs stream it in; post-exp work trails the
exp pipeline by two tiles so the in-order PE queue never waits on Act.
"""

import collections
import math

import numpy as np

import concourse.bass_isa as bass_isa
import concourse.mybir as mybir
import concourse.tile as tile
from concourse import bacc
from concourse.bass_utils import run_bass_kernel_spmd

B, T, C = 4, 4096, 128
P = 128            # partition width / head dim / key chunk
QB = 512           # query block
NQB = T // QB      # 8 query blocks
NKC = T // P // 2  # 16 key chunks per parity class

BF16 = mybir.dt.bfloat16
F32 = mybir.dt.float32

# ---- packed input layout (columns of in_all [128, TOTAL_COLS]) ----
# Z = (Wq^T Wk) Xk^T and V-chunks (v[k,d]) are precomputed on the HOST
# (memory-regime: trade a little extra DMA for zero on-device staging).
# Columns are laid out in consumption order for qblock order [2..7, 0, 1]
# (qblock 1 last: its final tile is the tiny diag exp, so most of the
# output tail drains before the last activation).
XT_OFF = [7424, 7936, 6912, 6400, 5888, 5376, 4864, 512]
MASK_OFF = 2560
TOTAL_COLS = 8448
DMA_SLICES = [
    (0, 1024), (1024, 2560), (2560, 3584), (3584, 4864), (4864, 5888),
    (5888, 6912), (6912, 7936), (7936, 8448),
]
QORDER = [7, 6, 5, 4, 3, 2, 0, 1]


def _z_off(c):
    return c * P if c < 4 else 1024 + (c - 4) * P


def _v_off(c):
    return 2816 + c * P


def build_kernel(cfg=None):
    base = dict(s_bufs=2, u_bufs=2, ws_bufs=2, ar_bufs=4, o_bufs=4, depth=2)
    base.update(cfg or {})
    cfg = base
    nc = bacc.Bacc(None, target_bir_lowering=False)

    in_all = nc.dram_tensor("in_all", [P, TOTAL_COLS], BF16, kind="ExternalInput")
    ou = nc.dram_tensor("ou", [P, T], F32, kind="ExternalOutput")
    se = nc.dram_tensor("se", [NQB, QB], F32, kind="ExternalOutput")

    scale = 1.0 / math.sqrt(C)
    WCOLS = 15 * QB + 256  # widest qblock: 15 full chunks + diag half

    with tile.TileContext(nc) as tc:
        with (
            tc.tile_pool(name="const", bufs=1) as const,
            tc.tile_pool(name="wspool", bufs=cfg["ws_bufs"]) as wspool,
            tc.tile_pool(name="arpool", bufs=cfg["ar_bufs"]) as arpool,
            tc.tile_pool(name="opool", bufs=cfg["o_bufs"]) as opool,
            tc.tile_pool(name="ps_s", bufs=cfg["s_bufs"], space="PSUM") as ps_s,
            tc.tile_pool(name="ps_u", bufs=cfg["u_bufs"], space="PSUM") as ps_u,
        ):
            in_sb = const.tile([P, TOTAL_COLS], BF16, tag="in_sb")
            w_bufs = [
                const.tile([P, WCOLS], BF16, name="w0", tag="w0"),
                const.tile([P, WCOLS], BF16, name="w1", tag="w1"),
            ]

            mask = in_sb[:, MASK_OFF : MASK_OFF + 256]

            for a, b in DMA_SLICES:
                nc.sync.dma_start(in_sb[:, a:b], in_all[:, a:b])

            # Warm the PE p-state during the input-DMA dead time: ~3us of
            # continuous dummy matmuls bring the clock to full speed before
            # the first real score matmul.
            wsrc = const.tile([P, QB], BF16, tag="wsrc")
            nc.vector.memset(wsrc[:], 0.0)
            warm = ps_s.tile([P, 1536], F32, name="warm", tag="ps")
            for _ in range(5):
                nc.tensor.matmul(
                    warm[:, :QB], wsrc[:, :P], wsrc[:],
                    start=True, stop=True,
                )

            def xt(i):
                return in_sb[:, XT_OFF[i] : XT_OFF[i] + QB]

            def z_chunk(c):
                return in_sb[:, _z_off(c) : _z_off(c) + P]

            def v_chunk(c):
                return in_sb[:, _v_off(c) : _v_off(c) + P]

            # Post-exp work (mask, ou-accumulate, running sum) for a score
            # tile trails the matmul+exp stream by `depth` tiles, so the
            # in-order PE/DVE queues never wait on the Act engine.  Each
            # qblock's diagonal+outputs ride the deque as its last closure.
            pend = collections.deque()
            held = []

            def push(fn):
                pend.append(fn)
                while len(pend) > cfg["depth"]:
                    pend.popleft()()

            for oi, i in enumerate(QORDER):
                nf = 2 * i + 1          # full-width chunks (incl. mask_lo chunk)
                packs, rem = nf // 3, nf % 3
                qs = slice(i * QB, (i + 1) * QB)
                w = w_bufs[oi % 2]
                xq = xt(i)
                psu = ps_u.tile([P, QB], F32, tag="psu")
                acc = None if i == 0 else wspool.tile([P, QB], BF16, tag="acc")
                state = {"acc_init": False}

                def wsl(c, o=0, e=QB, w=w):
                    return w[:, c * QB + o : c * QB + e]

                def post_exp(cs, i=i, psu=psu, acc=acc, state=state, wsl=wsl):
                    for c in cs:
                        if c == 2 * i:
                            nc.vector.tensor_mul(
                                out=wsl(c, 0, 256), in0=wsl(c, 0, 256), in1=mask
                            )
                        nc.tensor.matmul(
                            psu[:], v_chunk(c), wsl(c),
                            start=(c == 0), stop=False,
                        )
                        if acc is not None:
                            if not state["acc_init"]:
                                if c == 1:
                                    nc.vector.tensor_add(
                                        out=acc[:], in0=wsl(0), in1=wsl(1)
                                    )
                                    state["acc_init"] = True
                            else:
                                nc.vector.tensor_add(
                                    out=acc[:], in0=acc[:], in1=wsl(c)
                                )

                for j in range(packs):
                    pt = ps_s.tile([P, 1536], F32, tag="ps")
                    for k in range(3):
                        nc.tensor.matmul(
                            pt[:, k * QB : (k + 1) * QB], z_chunk(3 * j + k),
                            xq, start=True, stop=True,
                        )
                    nc.scalar.activation(
                        w[:, j * 1536 : (j + 1) * 1536], pt[:],
                        mybir.ActivationFunctionType.Exp, scale=scale,
                    )
                    push(lambda j=j, f=post_exp: f([3 * j, 3 * j + 1, 3 * j + 2]))

                # remainder fulls + diagonal half in one PSUM tile/activation
                pt = ps_s.tile([P, 1536], F32, tag="ps")
                for k in range(rem):
                    nc.tensor.matmul(
                        pt[:, k * QB : (k + 1) * QB], z_chunk(3 * packs + k),
                        xq, start=True, stop=True,
                    )
                nc.tensor.matmul(
                    pt[:, rem * QB : rem * QB + 256], z_chunk(nf),
                    xq[:, 256:], start=True, stop=True,
                )
                nc.scalar.activation(
                    w[:, packs * 1536 : packs * 1536 + rem * QB + 256],
                    pt[:, : rem * QB + 256],
                    mybir.ActivationFunctionType.Exp, scale=scale,
                )

                def epilogue(i=i, nf=nf, rem=rem, packs=packs, qs=qs, oi=oi,
                             psu=psu, acc=acc, post_exp=post_exp, wsl=wsl, w=w):
                    post_exp([3 * packs + k for k in range(rem)])
                    dsl = w[:, nf * QB : nf * QB + 256]
                    if oi == NQB - 1:
                        # Final qblock: nothing left to hide the tail behind,
                        # so drain both outputs in independent halves spread
                        # over idle queues.  Cols [0:256) of se/ou are final
                        # before the diagonal lands; the diag chain only
                        # gates [256:).
                        st = acc if acc is not None else wsl(0)
                        ar = arpool.tile([P, QB], F32, tag="ar")
                        o_sb = opool.tile([P, QB], F32, tag="o_sb")
                        nc.gpsimd.partition_all_reduce(
                            ar[:, :256], st[:, :256], channels=P,
                            reduce_op=bass_isa.ReduceOp.add,
                        )
                        nc.sync.dma_start(se[i : i + 1, :256], ar[0:1, :256])
                        # diagonal half: the se critical chain goes first;
                        # the ou evacuations ride behind it on other queues
                        nc.vector.tensor_mul(out=dsl, in0=dsl, in1=mask)
                        nc.tensor.matmul(
                            psu[:, 256:], v_chunk(nf), dsl,
                            start=False, stop=True,
                        )
                        nc.vector.tensor_add(
                            out=st[:, 256:], in0=st[:, 256:], in1=dsl
                        )
                        nc.gpsimd.partition_all_reduce(
                            ar[:, 256:], st[:, 256:], channels=P,
                            reduce_op=bass_isa.ReduceOp.add,
                        )
                        nc.sync.dma_start(se[i : i + 1, 256:], ar[0:1, 256:])
                        nc.vector.tensor_copy(out=o_sb[:, :256], in_=psu[:, :256])
                        nc.scalar.dma_start(
                            ou[:, i * QB : i * QB + 256], o_sb[:, :256]
                        )
                        nc.vector.tensor_copy(out=o_sb[:, 256:], in_=psu[:, 256:])
                        nc.scalar.dma_start(
                            ou[:, i * QB + 256 : (i + 1) * QB], o_sb[:, 256:]
                        )
                        return
                    # diagonal: mask, ou-accumulate cols [256:), add into sum
                    nc.vector.tensor_mul(out=dsl, in0=dsl, in1=mask)
                    nc.tensor.matmul(
                        psu[:, 256:], v_chunk(nf), dsl,
                        start=False, stop=True,
                    )
                    if acc is None:
                        nc.vector.tensor_add(
                            out=wsl(0, 256), in0=wsl(0, 256), in1=dsl
                        )
                        sum_tile = wsl(0)
                    else:
                        nc.vector.tensor_add(
                            out=acc[:, 256:], in0=acc[:, 256:], in1=dsl
                        )
                        sum_tile = acc[:]

                    ar = arpool.tile([P, QB], F32, tag="ar")
                    nc.gpsimd.partition_all_reduce(
                        ar[:], sum_tile, channels=P,
                        reduce_op=bass_isa.ReduceOp.add,
                    )
                    nc.sync.dma_start(se[i : i + 1, :], ar[0:1, :])
                    o_sb = opool.tile([P, QB], F32, tag="o_sb")
                    # gpsimd cannot access PSUM on hw; DVE does evacuations
                    nc.vector.tensor_copy(out=o_sb[:], in_=psu[:])
                    nc.sync.dma_start(ou[:, qs], o_sb[:])

                push(epilogue)
            while pend:
                pend.popleft()()
            for fn in held:
                fn()

    nc.compile()
    return nc


_NC_CACHE = {}


def _get_nc():
    if "nc" not in _NC_CACHE:
        _NC_CACHE["nc"] = build_kernel()
    return _NC_CACHE["nc"]


_STATIC = {}


def _static_parts(h):
    if h not in _STATIC:
        rows = np.concatenate(
            [np.arange(j * P, (j + 1) * P) for j in range(h, T // P, 2)]
        )
        s = np.arange(P)[:, None]
        q = np.arange(256)[None, :]
        _STATIC[h] = (rows, (q >= s + P * h).astype(np.float32))
    return _STATIC[h]


def _core_inputs(xb, Wq, Wk, Wv, h):
    """Build the packed input map for one core (batch xb [T,C], parity h)."""
    bf = mybir.dt.np(BF16)
    rows, mask = _static_parts(h)
    xk = xb[rows]                                  # [NKC*P, C]
    z = (Wq.T @ Wk) @ xk.T                         # [C, keys]
    v = (xk @ Wv.T).T                              # [C, keys]; v_chunk = v[:,cs].T
    xT = xb.T                                      # [C, T]
    pack = np.empty((P, TOTAL_COLS), dtype=bf)
    pack[:, MASK_OFF : MASK_OFF + 256] = mask.astype(bf)
    for i in range(NQB):
        pack[:, XT_OFF[i] : XT_OFF[i] + QB] = xT[:, i * QB : (i + 1) * QB].astype(bf)
    for c in range(NKC):
        cs = slice(c * P, (c + 1) * P)
        pack[:, _z_off(c) : _z_off(c) + P] = z[:, cs].astype(bf)
        pack[:, _v_off(c) : _v_off(c) + P] = v[:, cs].T.astype(bf)
    return {"in_all": pack}


def _build_runner(nc):
    """Cacheable PJRT runner (same machinery as bass2jax.run_bass_via_pjrt,
    but the jitted executable is built once and reused across kernel()
    calls instead of being re-traced every time)."""
    import jax
    from jax.sharding import Mesh, PartitionSpec
    from jax.experimental.shard_map import shard_map
    from concourse.bass2jax import (
        _bass_exec_p, install_neuronx_cc_hook, partition_id_tensor,
    )

    install_neuronx_cc_hook()
    pname = nc.partition_id_tensor.name if nc.partition_id_tensor else None
    in_names, out_names, out_avals, out_shapes = [], [], [], []
    for alloc in nc.m.functions[0].allocations:
        if not isinstance(alloc, mybir.MemoryLocationSet):
            continue
        name = alloc.memorylocations[0].name
        if alloc.kind == "ExternalInput":
            if name != pname:
                in_names.append(name)
        elif alloc.kind == "ExternalOutput":
            shape = tuple(alloc.tensor_shape)
            dtype = mybir.dt.np(alloc.dtype)
            out_names.append(name)
            out_avals.append(jax.core.ShapedArray(shape, dtype))
            out_shapes.append((shape, dtype))
    n_params, n_outs = len(in_names), len(out_avals)
    all_in = in_names + out_names + ([pname] if pname else [])
    donate = tuple(range(n_params, n_params + n_outs))

    def _body(*args):
        operands = list(args)
        if pname is not None:
            operands.append(partition_id_tensor())
        return tuple(
            _bass_exec_p.bind(
                *operands,
                out_avals=tuple(out_avals),
                in_names=tuple(all_in),
                out_names=tuple(out_names),
                lowering_input_output_aliases=(),
                sim_require_finite=True,
                sim_require_nnan=True,
                nc=nc,
            )
        )

    devices = jax.devices()[:8]
    mesh = Mesh(np.asarray(devices), ("core",))
    sharded = jax.jit(
        shard_map(
            _body, mesh=mesh,
            in_specs=(PartitionSpec("core"),) * (n_params + n_outs),
            out_specs=(PartitionSpec("core"),) * n_outs,
            check_rep=False,
        ),
        donate_argnums=donate, keep_unused=True,
    )

    def run(in_maps):
        concat_in = [
            np.concatenate([np.asarray(m[nm]) for m in in_maps], axis=0)
            for nm in in_names
        ]
        zeros = [
            np.zeros((8 * s[0],) + s[1:], d) for s, d in out_shapes
        ]
        outs = sharded(*concat_in, *zeros)
        return [
            {
                nm: np.asarray(outs[j]).reshape(8, *out_shapes[j][0])[c]
                for j, nm in enumerate(out_names)
            }
            for c in range(8)
        ]

    return run


def kernel(x, Wq, Wk, Wv, _trace=False):
    x = np.asarray(x, dtype=np.float32)
    Wq = np.asarray(Wq, dtype=np.float32)
    Wk = np.asarray(Wk, dtype=np.float32)
    Wv = np.asarray(Wv, dtype=np.float32)

    nc = _get_nc()
    in_maps = [_core_inputs(x[c // 2], Wq, Wk, Wv, c % 2) for c in range(8)]
    results = None
    if not _trace:
        try:
            if "runner" not in _NC_CACHE:
                _NC_CACHE["runner"] = _build_runner(nc)
            results = _NC_CACHE["runner"](in_maps)
        except Exception:
            _NC_CACHE.pop("runner", None)
            results = None
    if results is None:
        try:
            res = run_bass_kernel_spmd(
                nc, in_maps, core_ids=list(range(8)), trace=_trace
            )
        except ModuleNotFoundError:
            # axon NTFF profiling hook unavailable in this container
            res = run_bass_kernel_spmd(nc, in_maps, core_ids=list(range(8)))
        if _trace:
            _NC_CACHE["last_results"] = res
        results = res.results

    out = np.empty((B, T, C), dtype=np.float32)
    for b in range(B):
        a, bb = results[2 * b], results[2 * b + 1]
        denom = a["se"].reshape(T) + bb["se"].reshape(T)
        out[b] = ((a["ou"].astype(np.float32) + bb["ou"].astype(np.float32))
                  / denom[None, :]).T
    return out
